# revision 1
# baseline (speedup 1.0000x reference)
"""Trainium2 Bass kernel for nn_Attention (GQA + RoPE + softmax-n + causal).

Full inputs -> shard DP2(batch) x TP4(heads) across 8 cores -> gather+sum.

Per-core device program (all matmuls fp32r, PSUM fp32):
  phase 1: Q^T/K^T/V^T = w.T @ x^T   (x^T streamed in 512-col chunks)
           RoPE on Q^T/K^T via sign-folded tables + DMA partition half-swap
           V^T transposed back to natural V via PE transpose
  phase 2: per q-chunk (512 cols), per head:
           scores^T[k,q] = K^T.T @ Q^T  (causal: N-sliced bands)
           E = exp(scores^T)  (softmax-n: no max subtraction; scores ~N(0,0.8))
           diag 128x128 blocks masked by multiplying a triangle mask
           denom[1,q] = ones.T @ E (+1 phantom logit), accumulated in PSUM
           out^T[hd,q] += V.T @ E ;  out^T *= broadcast(1/denom)
           then output projection for this q-chunk: out += oc.T @ wo_shard

Host: out[b] = sum over 4 TP shards of out_partial.
"""
import sys
import numpy as np

sys.path.insert(0, "/opt/trn_rl_repo")

import concourse.bass as bass
import concourse.bacc as bacc
import concourse.mybir as mybir
import concourse.tile as tile
from concourse import bass_utils
from concourse._compat import with_exitstack

F32 = mybir.dt.float32
F32R = mybir.dt.float32r
EXP = mybir.ActivationFunctionType.Exp

B, S, D = 2, 2048, 2048
N_HEADS, N_KV_HEADS, HD = 16, 8, 128
TP = 4                      # tensor-parallel ways (x DP2 over batch = 8 cores)
QF = 4 * HD                 # per-core q feature cols   (512)
KF = 2 * HD                 # per-core k/v feature cols (256)
NQT = S // 128              # 16 seq tiles
NQC = S // 512              # 4  q-chunks
ND = D // 128               # 16 contraction tiles
NSC = S // 512              # 4  x^T stream chunks

_CACHE = {}

# matmul dtype config: "f32r" or "bf16" per stage
import os
CFG = {"proj": os.environ.get("K_PROJ", "f32r"),
       "attn": os.environ.get("K_ATTN", "f32r"),
       "wo": os.environ.get("K_WO", "f32r")}


def _dt(stage):
    return F32R if CFG[stage] == "f32r" else mybir.dt.bfloat16


def _npdt(stage):
    import ml_dtypes
    return np.float32 if CFG[stage] == "f32r" else ml_dtypes.bfloat16


def _build(bench_reps=None):
    nc = bacc.Bacc("TRN2", target_bir_lowering=False, debug=False)

    PJ, AT, WD = _dt("proj"), _dt("attn"), _dt("wo")
    names = [("xT", [D, S], PJ), ("wq", [D, QF], PJ), ("wk", [D, KF], PJ),
             ("wv", [D, KF], PJ), ("wo", [QF, D], WD),
             ("c2", [128, S], F32), ("g", [128, S], F32),
             ("tri", [128, 128], AT), ("tri2", [128, 256], AT),
             ("ones128", [128, 1], AT),
             ("onesrow", [1, 128], F32R), ("one1", [1, 1], AT),
             ("onerow512", [1, 512], AT), ("idn", [128, 128], AT),
             ("tri_b", [128, 128], mybir.dt.bfloat16),
             ("tri2_b", [128, 256], mybir.dt.bfloat16),
             ("ones128_b", [128, 1], mybir.dt.bfloat16),
             ("one1_b", [1, 1], mybir.dt.bfloat16),
             ("onerow512_b", [1, 512], mybir.dt.bfloat16)]
    kind = "Internal" if bench_reps else "ExternalInput"
    io = {n: nc.dram_tensor(n, sh, dt, kind=kind) for n, sh, dt in names}
    if bench_reps:
        io["dummy"] = nc.dram_tensor("bench_in", [128, 1], F32,
                                     kind="ExternalInput")
    io["out"] = nc.dram_tensor("out", [S, D], F32, kind="ExternalOutput")
    if os.environ.get("K_DEBUG"):
        io["dbg_qk"] = nc.dram_tensor("dbg_qk", [6 * 128, S], F32,
                                      kind="ExternalOutput")
        io["dbg_v"] = nc.dram_tensor("dbg_v", [128, KF], F32,
                                     kind="ExternalOutput")

    with tile.TileContext(nc) as tc:
        if bench_reps:
            # fill internal DRAM inputs with benign constants (avoid
            # garbage -> denormal/NaN timing artifacts)
            with tc.tile_pool(name="fillp", bufs=1) as fp:
                f3t = fp.tile([128, 2048], F32, tag="fill32")
                fbt = fp.tile([128, 2048], mybir.dt.bfloat16, tag="fillb")
                nc.gpsimd.memset(f3t[:], 0.001)
                nc.gpsimd.memset(fbt[:], 0.001)
                for n, sh, dt in names:
                    r, c = sh
                    for r0 in range(0, r, 128):
                        rr = min(128, r - r0)
                        for c0 in range(0, c, 2048):
                            cc = min(2048, c - c0)
                            if dt == mybir.dt.bfloat16:
                                srcap = fbt[:rr, :cc]
                            elif dt == F32R:
                                srcap = f3t[:rr, :cc].bitcast(F32R)
                            else:
                                srcap = f3t[:rr, :cc]
                            nc.sync.dma_start(io[n][r0:r0 + rr, c0:c0 + cc],
                                              srcap)
        if bench_reps and bench_reps > 1:
            with tc.For_i(0, bench_reps, 1):
                _emit(tc, nc, io)
        else:
            _emit(tc, nc, io)
    nc.compile()
    return nc


@with_exitstack
def _emit(ctx, tc, nc, io):
    ts = bass.ts
    PJ, AT, WD = _dt("proj"), _dt("attn"), _dt("wo")
    persist = ctx.enter_context(tc.tile_pool(name="persist", bufs=1))

    # ---- persistent SBUF tensors (live whole kernel) ----
    tri = persist.tile([128, 128], AT, tag="tri")
    tri2 = persist.tile([128, 256], AT, tag="tri2")
    ones128 = persist.tile([128, 1], AT, tag="ones128")
    onesrow = persist.tile([1, 128], F32R, tag="onesrow")
    one1 = persist.tile([1, 1], AT, tag="one1")
    onerow512 = persist.tile([1, 512], AT, tag="onerow512")
    tri_b = persist.tile([128, 128], mybir.dt.bfloat16, tag="tri_b")
    tri2_b = persist.tile([128, 256], mybir.dt.bfloat16, tag="tri2_b")
    ones128_b = persist.tile([128, 1], mybir.dt.bfloat16, tag="ones128_b")
    one1_b = persist.tile([1, 1], mybir.dt.bfloat16, tag="one1_b")
    onerow512_b = persist.tile([1, 512], mybir.dt.bfloat16, tag="onerow512_b")
    for name, t in [("tri", tri), ("tri2", tri2), ("ones128", ones128),
                    ("onesrow", onesrow), ("one1", one1),
                    ("onerow512", onerow512), ("tri_b", tri_b),
                    ("tri2_b", tri2_b), ("ones128_b", ones128_b),
                    ("one1_b", one1_b), ("onerow512_b", onerow512_b)]:
        nc.sync.dma_start(t[:], io[name][:])

    # rotated Q^T/K^T: 6 head tiles [128, S]; V natural: 16 tiles [128, KF]
    qkT = [persist.tile([128, S], AT, tag=f"qkT{f}", name=f"qkT{f}")
           for f in range(6)]
    vnat = [persist.tile([128, KF], AT, tag=f"vnat{st}", name=f"vnat{st}")
            for st in range(NQT)]
    # ================= phase 1: projections + rope + V transpose ==========
    with tc.tile_pool(name="wp", bufs=1) as wp, \
         tc.tile_pool(name="xtp", bufs=21) as xtp, \
         tc.tile_pool(name="cgp", bufs=2) as cgp, \
         tc.tile_pool(name="rope", bufs=3) as ropep, \
         tc.tile_pool(name="vsb", bufs=2) as vsbp, \
         tc.tile_pool(name="p1ps", bufs=4, space="PSUM") as p1ps, \
         tc.tile_pool(name="vtps", bufs=2, space="PSUM") as vtps:
        idn = wp.tile([128, 128], AT, tag="idn")
        nc.sync.dma_start(idn[:], io["idn"][:])
        wq_sb = [wp.tile([128, QF], PJ, tag=f"wq{d}", name=f"wq{d}")
                 for d in range(ND)]
        wk_sb = [wp.tile([128, KF], PJ, tag=f"wk{d}", name=f"wk{d}")
                 for d in range(ND)]
        wv_sb = [wp.tile([128, KF], PJ, tag=f"wv{d}", name=f"wv{d}")
                 for d in range(ND)]
        # interleave wq with x^T chunk 0 so the first matmul chain starts
        # as early as possible; wk/wv (needed later) load after.
        xt0 = []
        for d in range(ND):
            nc.sync.dma_start(wq_sb[d][:], io["wq"][ts(d, 128), :])
            t = xtp.tile([128, 512], PJ, tag="xt", name=f"xt0_{d}")
            nc.scalar.dma_start(t[:], io["xT"][ts(d, 128), 0:512])
            xt0.append(t)
        for d in range(ND):
            nc.sync.dma_start(wk_sb[d][:], io["wk"][ts(d, 128), :])
            nc.sync.dma_start(wv_sb[d][:], io["wv"][ts(d, 128), :])

        for sc in range(NSC):                    # 512-wide x^T chunks
            cs = ts(sc, 512)
            if sc == 0:
                xt = xt0
            else:
                xt = []
                for d in range(ND):
                    t = xtp.tile([128, 512], PJ, tag="xt", name=f"xt{sc}_{d}")
                    nc.scalar.dma_start(t[:], io["xT"][ts(d, 128), cs])
                    xt.append(t)
            c2c = cgp.tile([128, 512], F32, tag="c2c")
            gc = cgp.tile([128, 512], F32, tag="gc")
            nc.gpsimd.dma_start(c2c[:], io["c2"][:, cs])
            nc.gpsimd.dma_start(gc[:], io["g"][:, cs])
            # f: 0..3 q-heads, 4..5 k-heads, 6..7 v-heads
            for f in range(8):
                if f < 4:
                    wt, fo = wq_sb, f * 128
                elif f < 6:
                    wt, fo = wk_sb, (f - 4) * 128
                else:
                    wt, fo = wv_sb, (f - 6) * 128
                ps = p1ps.tile([128, 512], F32, tag="proj")
                for d in range(ND):
                    nc.tensor.matmul(ps[:], wt[d][:, fo:fo + 128], xt[d][:],
                                     start=(d == 0), stop=(d == ND - 1))
                if f < 6:
                    # rope: rot = ps*c2 + halfswap(ps*g)
                    a = ropep.tile([128, 512], F32, tag="ropeA")
                    b = ropep.tile([128, 512], F32, tag="ropeB")
                    bsw = ropep.tile([128, 512], F32, tag="ropeBsw")
                    nc.vector.tensor_mul(a[:], ps[:], c2c[:])
                    nc.vector.tensor_mul(b[:], ps[:], gc[:])
                    nc.gpsimd.dma_start(bsw[0:64, :], b[64:128, :])
                    nc.gpsimd.dma_start(bsw[64:128, :], b[0:64, :])
                    if AT == F32R:
                        nc.vector.tensor_add(qkT[f][:, cs], a[:], bsw[:])
                    else:
                        rotf = ropep.tile([128, 512], F32, tag="rotf")
                        nc.vector.tensor_add(rotf[:], a[:], bsw[:])
                        nc.vector.tensor_copy(qkT[f][:, cs], rotf[:])
                else:
                    # V^T -> copy to SBUF -> PE-transpose 128x128 blocks
                    vt = vsbp.tile([128, 512], AT, tag="vT")
                    nc.vector.tensor_copy(vt[:], ps[:])
                    for sub in range(4):
                        st = sc * 4 + sub
                        tp = vtps.tile([128, 128], AT, tag="vtp")
                        nc.tensor.transpose(tp[:], vt[:, ts(sub, 128)], idn[:])
                        nc.vector.tensor_copy(
                            vnat[st][:, (f - 6) * 128:(f - 5) * 128], tp[:])

    # ============ phase 2: attention + fused output projection ============
    with tc.tile_pool(name="ep", bufs=8) as ep, \
         tc.tile_pool(name="ocp", bufs=3) as ocp, \
         tc.tile_pool(name="fin", bufs=2) as finp, \
         tc.tile_pool(name="osb", bufs=6) as osbp, \
         tc.tile_pool(name="scps", bufs=3, space="PSUM") as scps, \
         tc.tile_pool(name="outps", bufs=2, space="PSUM") as outps, \
         tc.tile_pool(name="denps", bufs=1, space="PSUM") as denps, \
         tc.tile_pool(name="w3ps", bufs=2, space="PSUM") as w3ps, \
         tc.tile_pool(name="wop", bufs=1) as wop:
        wo_sb = [wop.tile([128, D], WD, tag=f"wo{hf}", name=f"wo{hf}")
                 for hf in range(4)]
        for hf in range(4):
            nc.sync.dma_start(wo_sb[hf][:], io["wo"][ts(hf, 128), :])
        for qc in range(NQC):
            qs = qc * 512
            oc = []
            for h in range(4):
                gkv = h // 2
                qT, kT = qkT[h], qkT[4 + gkv]
                out_ps = outps.tile([128, 512], F32, tag="out")
                den_ps = denps.tile([1, 512], F32, tag="den")
                # +1 phantom logit (softmax-n)
                nc.tensor.matmul(den_ps[:], one1[:], onerow512[:],
                                 start=True, stop=False)
                nkt = 4 * (qc + 1)
                for kt in range(nkt):
                    off = max(0, 128 * kt - qs)
                    diag = kt >= 4 * qc
                    moff = off
                    if off == 384:
                        off = 256        # keep N>=256 (fp32r full rate)
                    sc_ps = scps.tile([128, 512], F32, tag="sc")
                    nc.tensor.matmul(sc_ps[:, off:], kT[:, ts(kt, 128)],
                                     qT[:, qs + off:qs + 512],
                                     start=True, stop=True)
                    e = ep.tile([128, 512], AT, tag="e")
                    nc.scalar.activation(e[:, off:], sc_ps[:, off:], EXP)
                    if diag:
                        if moff == 384:
                            nc.vector.tensor_mul(e[:, 256:512],
                                                 e[:, 256:512], tri2[:])
                        else:
                            nc.vector.tensor_mul(e[:, moff:moff + 128],
                                                 e[:, moff:moff + 128], tri[:])
                    nc.tensor.matmul(out_ps[:, off:],
                                     vnat[kt][:, gkv * 128:(gkv + 1) * 128],
                                     e[:, off:],
                                     start=(kt == 0), stop=(kt == nkt - 1))
                    nc.tensor.matmul(den_ps[:, off:], ones128[:], e[:, off:],
                                     start=False, stop=(kt == nkt - 1))
                rec = finp.tile([1, 512], F32, tag="rec")
                with nc.allow_low_precision(reason="recip of denom"):
                    nc.vector.reciprocal(rec[:], den_ps[:])
                bcs = finp.tile([128, 512], F32, tag="bcs")
                nc.gpsimd.partition_broadcast(bcs[:], rec[:])
                o = ocp.tile([128, 512], WD, tag=f"oc{h}", name=f"oc{h}_{qc}")
                if WD == F32R:
                    nc.vector.tensor_mul(o[:], out_ps[:], bcs[:])
                else:
                    of = finp.tile([128, 512], F32, tag="ocf")
                    nc.vector.tensor_mul(of[:], out_ps[:], bcs[:])
                    nc.vector.tensor_copy(o[:], of[:])
                oc.append(o)
            # fused output projection for this q-chunk's 4 seq tiles
            for sub in range(4):
                st = qc * 4 + sub
                for dc in range(4):
                    ps3 = w3ps.tile([128, 512], F32, tag="wo3")
                    for hf in range(4):
                        nc.tensor.matmul(ps3[:], oc[hf][:, ts(sub, 128)],
                                         wo_sb[hf][:, ts(dc, 512)],
                                         start=(hf == 0), stop=(hf == 3))
                    o3 = osbp.tile([128, 512], F32, tag="o3")
                    nc.vector.tensor_copy(o3[:], ps3[:])
                    nc.sync.dma_start(io["out"][ts(st, 128), ts(dc, 512)], o3[:])
        if "dbg_qk" in io:
            with tc.tile_pool(name="dbgp", bufs=2) as dbgp:
                for f in range(6):
                    dt_ = dbgp.tile([128, S], F32, tag="dbg")
                    nc.vector.tensor_copy(dt_[:], qkT[f][:])
                    nc.sync.dma_start(io["dbg_qk"][ts(f, 128), :], dt_[:])
                dv = dbgp.tile([128, KF], F32, tag="dbgv")
                nc.vector.tensor_copy(dv[:], vnat[0][:])
                nc.sync.dma_start(io["dbg_v"][:], dv[:])


def _host_prep(x, freqs_cos, freqs_sin, wq, wk, wv, wo):
    """Build the 8 per-core input maps."""
    # de-interleave perm within every 128-col head block: [0,2,..,126,1,3,..,127]
    p128 = np.concatenate([np.arange(0, 128, 2), np.arange(1, 128, 2)])
    permq = np.concatenate([hb * 128 + p128 for hb in range(N_HEADS)])
    permk = np.concatenate([hb * 128 + p128 for hb in range(N_KV_HEADS)])
    wq_p = (wq / np.sqrt(np.float32(HD)))[:, permq]
    wk_p = wk[:, permk]

    cosT = np.ascontiguousarray(freqs_cos.T)            # [64, S]
    sinT = np.ascontiguousarray(freqs_sin.T)
    c2 = np.concatenate([cosT, cosT], 0).astype(np.float32)   # [128, S]
    gtab = np.concatenate([sinT, -sinT], 0).astype(np.float32)

    ii, jj = np.meshgrid(np.arange(128), np.arange(128), indexing="ij")
    tri = (ii <= jj).astype(np.float32)                 # [k, q] allow k<=q

    tri2 = np.concatenate([np.zeros((128, 128), np.float32), tri], 1)
    at, pj, wd = _npdt("attn"), _npdt("proj"), _npdt("wo")
    common = {
        "c2": c2, "g": gtab, "tri": tri.astype(at), "tri2": tri2.astype(at),
        "ones128": np.ones((128, 1), at),
        "onesrow": np.ones((1, 128), np.float32),
        "one1": np.ones((1, 1), at),
        "onerow512": np.ones((1, 512), at),
        "idn": np.eye(128, dtype=at),
    }
    import ml_dtypes
    common.update({
        "tri_b": tri.astype(ml_dtypes.bfloat16),
        "tri2_b": tri2.astype(ml_dtypes.bfloat16),
        "ones128_b": np.ones((128, 1), ml_dtypes.bfloat16),
        "one1_b": np.ones((1, 1), ml_dtypes.bfloat16),
        "onerow512_b": np.ones((1, 512), ml_dtypes.bfloat16),
    })
    in_maps = []
    for core in range(8):
        b, t = divmod(core, TP)
        in_maps.append({
            "xT": np.ascontiguousarray(x[b].T).astype(pj),
            "wq": np.ascontiguousarray(wq_p[:, t * QF:(t + 1) * QF]).astype(pj),
            "wk": np.ascontiguousarray(wk_p[:, t * KF:(t + 1) * KF]).astype(pj),
            "wv": np.ascontiguousarray(wv[:, t * KF:(t + 1) * KF]).astype(pj),
            "wo": np.ascontiguousarray(wo[t * QF:(t + 1) * QF, :]).astype(wd),
            **common,
        })
    return in_maps


def kernel(x, freqs_cos, freqs_sin, wq, wk, wv, wo, _trace=False):
    in_maps = _host_prep(np.asarray(x, np.float32),
                         np.asarray(freqs_cos, np.float32),
                         np.asarray(freqs_sin, np.float32),
                         np.asarray(wq, np.float32), np.asarray(wk, np.float32),
                         np.asarray(wv, np.float32), np.asarray(wo, np.float32))
    if "nc" not in _CACHE:
        _CACHE["nc"] = _build()
    res = bass_utils.run_bass_kernel_spmd(_CACHE["nc"], in_maps, list(range(8)),
                                          trace=_trace)
    _CACHE["last_result"] = res
    out = np.zeros((B, S, D), np.float32)
    for core in range(8):
        b = core // TP
        out[b] += res.results[core]["out"]
    return out



# revision 8
# speedup vs baseline: 1.1930x; 1.1930x over previous
"""Trainium2 Bass kernel for nn_Attention (GQA + RoPE + softmax-n + causal).

Full inputs -> shard DP2(batch) x TP4(heads) across 8 cores -> gather+sum.

Per-core device program (all matmuls fp32r, PSUM fp32):
  phase 1: Q^T/K^T/V^T = w.T @ x^T   (x^T streamed in 512-col chunks)
           RoPE on Q^T/K^T via sign-folded tables + DMA partition half-swap
           V^T transposed back to natural V via PE transpose
  phase 2: per q-chunk (512 cols), per head:
           scores^T[k,q] = K^T.T @ Q^T  (causal: N-sliced bands)
           E = exp(scores^T)  (softmax-n: no max subtraction; scores ~N(0,0.8))
           diag 128x128 blocks masked by multiplying a triangle mask
           denom[1,q] = ones.T @ E (+1 phantom logit), accumulated in PSUM
           out^T[hd,q] += V.T @ E ;  out^T *= broadcast(1/denom)
           then output projection for this q-chunk: out += oc.T @ wo_shard

Host: out[b] = sum over 4 TP shards of out_partial.
"""
import sys
import numpy as np

sys.path.insert(0, "/opt/trn_rl_repo")

import concourse.bass as bass
import concourse.bacc as bacc
import concourse.mybir as mybir
import concourse.tile as tile
from concourse import bass_utils
from concourse._compat import with_exitstack

F32 = mybir.dt.float32
F32R = mybir.dt.float32r
EXP = mybir.ActivationFunctionType.Exp

B, S, D = 2, 2048, 2048
N_HEADS, N_KV_HEADS, HD = 16, 8, 128
TP = 4                      # tensor-parallel ways (x DP2 over batch = 8 cores)
QF = 4 * HD                 # per-core q feature cols   (512)
KF = 2 * HD                 # per-core k/v feature cols (256)
NQT = S // 128              # 16 seq tiles
NQC = S // 512              # 4  q-chunks
ND = D // 128               # 16 contraction tiles
NSC = S // 512              # 4  x^T stream chunks

_CACHE = {}

# matmul dtype config: "f32r" or "bf16" per stage
import os
CFG = {"proj": os.environ.get("K_PROJ", "f32r"),
       "attn": os.environ.get("K_ATTN", "f32r"),
       "wo": os.environ.get("K_WO", "f32r")}
CFG["attnq"] = os.environ.get("K_ATTNQ", CFG["attn"])
CFG["attne"] = os.environ.get("K_ATTNE", CFG["attn"])


def _dt(stage):
    return F32R if CFG[stage] == "f32r" else mybir.dt.bfloat16


def _npdt(stage):
    import ml_dtypes
    return np.float32 if CFG[stage] == "f32r" else ml_dtypes.bfloat16


def _build(bench_reps=None):
    nc = bacc.Bacc("TRN2", target_bir_lowering=False, debug=False)

    PJ, WD = _dt("proj"), _dt("wo")
    ATQ, ATE = _dt("attnq"), _dt("attne")
    names = [("xT", [D, S], PJ), ("wq", [D, QF], PJ), ("wk", [D, KF], PJ),
             ("wv", [D, KF], PJ), ("wo", [QF, D], WD),
             ("c2", [128, S], F32), ("g", [128, S], F32),
             ("tri", [128, 128], ATE), ("tri2", [128, 256], ATE),
             ("ones128", [128, 1], ATE),
             ("onesrow", [1, 128], F32R), ("one1", [1, 1], ATE),
             ("onerow512", [1, 512], ATE), ("idn", [128, 128], ATE),
             ("tri_b", [128, 128], mybir.dt.bfloat16),
             ("tri2_b", [128, 256], mybir.dt.bfloat16),
             ("ones128_b", [128, 1], mybir.dt.bfloat16),
             ("one1_b", [1, 1], mybir.dt.bfloat16),
             ("onerow512_b", [1, 512], mybir.dt.bfloat16)]
    kind = "Internal" if bench_reps else "ExternalInput"
    io = {n: nc.dram_tensor(n, sh, dt, kind=kind) for n, sh, dt in names}
    if bench_reps:
        io["dummy"] = nc.dram_tensor("bench_in", [128, 1], F32,
                                     kind="ExternalInput")
    io["out"] = nc.dram_tensor("out", [S, D], F32, kind="ExternalOutput")
    if os.environ.get("K_DEBUG"):
        io["dbg_qk"] = nc.dram_tensor("dbg_qk", [6 * 128, S], F32,
                                      kind="ExternalOutput")
        io["dbg_v"] = nc.dram_tensor("dbg_v", [128, KF], F32,
                                     kind="ExternalOutput")

    with tile.TileContext(nc) as tc:
        if bench_reps:
            # fill internal DRAM inputs with benign constants (avoid
            # garbage -> denormal/NaN timing artifacts)
            with tc.tile_pool(name="fillp", bufs=1) as fp:
                f3t = fp.tile([128, 2048], F32, tag="fill32")
                fbt = fp.tile([128, 2048], mybir.dt.bfloat16, tag="fillb")
                nc.gpsimd.memset(f3t[:], 0.001)
                nc.gpsimd.memset(fbt[:], 0.001)
                for n, sh, dt in names:
                    r, c = sh
                    for r0 in range(0, r, 128):
                        rr = min(128, r - r0)
                        for c0 in range(0, c, 2048):
                            cc = min(2048, c - c0)
                            if dt == mybir.dt.bfloat16:
                                srcap = fbt[:rr, :cc]
                            elif dt == F32R:
                                srcap = f3t[:rr, :cc].bitcast(F32R)
                            else:
                                srcap = f3t[:rr, :cc]
                            nc.sync.dma_start(io[n][r0:r0 + rr, c0:c0 + cc],
                                              srcap)
        if bench_reps and bench_reps > 1:
            with tc.For_i(0, bench_reps, 1):
                _emit(tc, nc, io)
        else:
            _emit(tc, nc, io)
    nc.compile()
    return nc


@with_exitstack
def _emit(ctx, tc, nc, io):
    ts = bass.ts
    PJ, WD = _dt("proj"), _dt("wo")
    ATQ, ATE = _dt("attnq"), _dt("attne")
    persist = ctx.enter_context(tc.tile_pool(name="persist", bufs=1))

    # ---- persistent SBUF tensors (live whole kernel) ----
    tri = persist.tile([128, 128], ATE, tag="tri")
    tri2 = persist.tile([128, 256], ATE, tag="tri2")
    ones128 = persist.tile([128, 1], ATE, tag="ones128")
    onesrow = persist.tile([1, 128], F32R, tag="onesrow")
    one1 = persist.tile([1, 1], ATE, tag="one1")
    onerow512 = persist.tile([1, 512], ATE, tag="onerow512")
    tri_b = persist.tile([128, 128], mybir.dt.bfloat16, tag="tri_b")
    tri2_b = persist.tile([128, 256], mybir.dt.bfloat16, tag="tri2_b")
    ones128_b = persist.tile([128, 1], mybir.dt.bfloat16, tag="ones128_b")
    one1_b = persist.tile([1, 1], mybir.dt.bfloat16, tag="one1_b")
    onerow512_b = persist.tile([1, 512], mybir.dt.bfloat16, tag="onerow512_b")
    for name, t in [("tri", tri), ("tri2", tri2), ("ones128", ones128),
                    ("onesrow", onesrow), ("one1", one1),
                    ("onerow512", onerow512), ("tri_b", tri_b),
                    ("tri2_b", tri2_b), ("ones128_b", ones128_b),
                    ("one1_b", one1_b), ("onerow512_b", onerow512_b)]:
        nc.sync.dma_start(t[:], io[name][:])

    # rotated Q^T/K^T: 6 head tiles [128, S]; V natural: 16 tiles [128, KF]
    qkT = [persist.tile([128, S], ATQ, tag=f"qkT{f}", name=f"qkT{f}")
           for f in range(6)]
    vnat = [persist.tile([128, KF], ATE, tag=f"vnat{st}", name=f"vnat{st}")
            for st in range(NQT)]
    PHASE = os.environ.get("K_PHASE", "")
    if PHASE == "2":
        # phase-2-only probe: fill qkT/vnat with small benign values
        wq_ = S if ATQ == F32R else S // 2
        we_ = KF if ATE == F32R else KF // 2
        for f in range(6):
            nc.sync.dma_start(qkT[f][:].bitcast(F32), io["c2"][:, :wq_])
        for st in range(NQT):
            nc.sync.dma_start(vnat[st][:].bitcast(F32), io["c2"][:, :we_])
    # ================= phase 1: projections + rope + V transpose ==========
    # x^T streamed in W1-wide chunks (matmul PSUM out is bank-limited: 512)
    W1 = 512
    NSC1 = S // W1
    NSUB = W1 // 128
    with tc.tile_pool(name="wp", bufs=1) as wp, \
         tc.tile_pool(name="xtp", bufs=21) as xtp, \
         tc.tile_pool(name="cgp", bufs=2) as cgp, \
         tc.tile_pool(name="rope", bufs=3) as ropep, \
         tc.tile_pool(name="vsb", bufs=2) as vsbp, \
         tc.tile_pool(name="p1ps", bufs=(3 if W1 == 1024 else 4),
                      space="PSUM") as p1ps, \
         tc.tile_pool(name="vtps", bufs=2, space="PSUM") as vtps:
        idn = wp.tile([128, 128], ATE, tag="idn")
        nc.sync.dma_start(idn[:], io["idn"][:])
        wq_sb = [wp.tile([128, QF], PJ, tag=f"wq{d}", name=f"wq{d}")
                 for d in range(ND)]
        wk_sb = [wp.tile([128, KF], PJ, tag=f"wk{d}", name=f"wk{d}")
                 for d in range(ND)]
        wv_sb = [wp.tile([128, KF], PJ, tag=f"wv{d}", name=f"wv{d}")
                 for d in range(ND)]
        # interleave wq with x^T chunk 0 so the first matmul chain starts
        # as early as possible; wk/wv (needed later) load after.
        xt0 = []
        for d in range(ND):
            nc.sync.dma_start(wq_sb[d][:], io["wq"][ts(d, 128), :])
            t = xtp.tile([128, W1], PJ, tag="xt", name=f"xt0_{d}")
            nc.scalar.dma_start(t[:], io["xT"][ts(d, 128), 0:W1])
            xt0.append(t)
        for d in range(ND):
            nc.sync.dma_start(wk_sb[d][:], io["wk"][ts(d, 128), :])
            nc.sync.dma_start(wv_sb[d][:], io["wv"][ts(d, 128), :])

        for sc in range(NSC1):                   # W1-wide x^T chunks
            cs = ts(sc, W1)
            if sc == 0:
                xt = xt0
            else:
                xt = []
                for d in range(ND):
                    t = xtp.tile([128, W1], PJ, tag="xt", name=f"xt{sc}_{d}")
                    nc.scalar.dma_start(t[:], io["xT"][ts(d, 128), cs])
                    xt.append(t)
            c2c = cgp.tile([128, W1], F32, tag="c2c")
            gc = cgp.tile([128, W1], F32, tag="gc")
            nc.sync.dma_start(c2c[:], io["c2"][:, cs])
            nc.sync.dma_start(gc[:], io["g"][:, cs])
            # f: 0..3 q-heads, 4..5 k-heads
            for f in range(6):
                if f < 4:
                    wt, fo = wq_sb, f * 128
                else:
                    wt, fo = wk_sb, (f - 4) * 128
                ps = p1ps.tile([128, W1], F32, tag="proj")
                for d in range(ND):
                    nc.tensor.matmul(ps[:], wt[d][:, fo:fo + 128], xt[d][:],
                                     start=(d == 0), stop=(d == ND - 1))
                # rope: rot = ps*c2 + halfswap(ps*g)
                a = ropep.tile([128, W1], F32, tag="ropeA")
                b = ropep.tile([128, W1], F32, tag="ropeB")
                bsw = ropep.tile([128, W1], F32, tag="ropeBsw")
                nc.vector.tensor_mul(a[:], ps[:], c2c[:])
                nc.vector.tensor_mul(b[:], ps[:], gc[:])
                nc.gpsimd.dma_start(bsw[0:64, :], b[64:128, :])
                nc.gpsimd.dma_start(bsw[64:128, :], b[0:64, :])
                nc.vector.tensor_add(qkT[f][:, cs], a[:], bsw[:])
            # V directly in natural [s, kv] layout: xt_slice.T @ wv
            for sub in range(NSUB):
                st = sc * NSUB + sub
                vps = vtps.tile([128, KF], F32, tag="vps")
                for d in range(ND):
                    nc.tensor.matmul(vps[:], xt[d][:, ts(sub, 128)],
                                     wv_sb[d][:],
                                     start=(d == 0), stop=(d == ND - 1))
                nc.vector.tensor_copy(vnat[st][:], vps[:])

    # ============ phase 2: attention + fused output projection ============
    with tc.tile_pool(name="ep", bufs=8) as ep, \
         tc.tile_pool(name="ocp", bufs=3) as ocp, \
         tc.tile_pool(name="fin", bufs=2) as finp, \
         tc.tile_pool(name="osb", bufs=6) as osbp, \
         tc.tile_pool(name="scps", bufs=3, space="PSUM") as scps, \
         tc.tile_pool(name="outps", bufs=2, space="PSUM") as outps, \
         tc.tile_pool(name="denps", bufs=1, space="PSUM") as denps, \
         tc.tile_pool(name="w3ps", bufs=2, space="PSUM") as w3ps, \
         tc.tile_pool(name="wop", bufs=1) as wop:
        wo_sb = [wop.tile([128, D], WD, tag=f"wo{hf}", name=f"wo{hf}")
                 for hf in range(4)]
        for hf in range(4):
            nc.sync.dma_start(wo_sb[hf][:], io["wo"][ts(hf, 128), :])
        for qc in range(NQC):
            qs = qc * 512
            oc = []
            for h in range(4):
                gkv = h // 2
                qT, kT = qkT[h], qkT[4 + gkv]
                out_ps = outps.tile([128, 512], F32, tag="out")
                den_ps = denps.tile([1, 512], F32, tag="den")
                # +1 phantom logit (softmax-n)
                nc.tensor.matmul(den_ps[:], one1[:], onerow512[:],
                                 start=True, stop=False)
                nkt = 4 * (qc + 1)
                for kt in range(nkt):
                    off = max(0, 128 * kt - qs)
                    diag = kt >= 4 * qc
                    moff = off
                    if off == 384 and ATQ == F32R:
                        off = 256        # keep N>=256 (fp32r full rate)
                    sc_ps = scps.tile([128, 512], F32, tag="sc")
                    nc.tensor.matmul(sc_ps[:, off:], kT[:, ts(kt, 128)],
                                     qT[:, qs + off:qs + 512],
                                     start=True, stop=True)
                    e = ep.tile([128, 512], ATE, tag="e")
                    nc.scalar.activation(e[:, off:], sc_ps[:, off:], EXP)
                    if diag:
                        if moff == 384:
                            nc.vector.tensor_mul(e[:, 256:512],
                                                 e[:, 256:512], tri2[:])
                        else:
                            nc.vector.tensor_mul(e[:, moff:moff + 128],
                                                 e[:, moff:moff + 128], tri[:])
                    nc.tensor.matmul(out_ps[:, off:],
                                     vnat[kt][:, gkv * 128:(gkv + 1) * 128],
                                     e[:, off:],
                                     start=(kt == 0), stop=(kt == nkt - 1))
                    nc.tensor.matmul(den_ps[:, off:], ones128[:], e[:, off:],
                                     start=False, stop=(kt == nkt - 1))
                rec = finp.tile([1, 512], F32, tag="rec")
                with nc.allow_low_precision(reason="recip of denom"):
                    nc.vector.reciprocal(rec[:], den_ps[:])
                bcs = finp.tile([128, 512], F32, tag="bcs")
                nc.gpsimd.partition_broadcast(bcs[:], rec[:])
                o = ocp.tile([128, 512], WD, tag=f"oc{h}", name=f"oc{h}_{qc}")
                nc.vector.tensor_mul(o[:], out_ps[:], bcs[:])
                oc.append(o)
            # fused output projection for this q-chunk's 4 seq tiles
            for sub in range(4):
                st = qc * 4 + sub
                for dc in range(4):
                    ps3 = w3ps.tile([128, 512], F32, tag="wo3")
                    for hf in range(4):
                        nc.tensor.matmul(ps3[:], oc[hf][:, ts(sub, 128)],
                                         wo_sb[hf][:, ts(dc, 512)],
                                         start=(hf == 0), stop=(hf == 3))
                    o3 = osbp.tile([128, 512], F32, tag="o3")
                    nc.vector.tensor_copy(o3[:], ps3[:])
                    nc.sync.dma_start(io["out"][ts(st, 128), ts(dc, 512)], o3[:])
        if "dbg_qk" in io:
            with tc.tile_pool(name="dbgp", bufs=2) as dbgp:
                for f in range(6):
                    dt_ = dbgp.tile([128, S], F32, tag="dbg")
                    nc.vector.tensor_copy(dt_[:], qkT[f][:])
                    nc.sync.dma_start(io["dbg_qk"][ts(f, 128), :], dt_[:])
                dv = dbgp.tile([128, KF], F32, tag="dbgv")
                nc.vector.tensor_copy(dv[:], vnat[0][:])
                nc.sync.dma_start(io["dbg_v"][:], dv[:])


def _host_prep(x, freqs_cos, freqs_sin, wq, wk, wv, wo):
    """Build the 8 per-core input maps."""
    # de-interleave perm within every 128-col head block: [0,2,..,126,1,3,..,127]
    p128 = np.concatenate([np.arange(0, 128, 2), np.arange(1, 128, 2)])
    permq = np.concatenate([hb * 128 + p128 for hb in range(N_HEADS)])
    permk = np.concatenate([hb * 128 + p128 for hb in range(N_KV_HEADS)])
    wq_p = (wq / np.sqrt(np.float32(HD)))[:, permq]
    wk_p = wk[:, permk]

    cosT = np.ascontiguousarray(freqs_cos.T)            # [64, S]
    sinT = np.ascontiguousarray(freqs_sin.T)
    c2 = np.concatenate([cosT, cosT], 0).astype(np.float32)   # [128, S]
    gtab = np.concatenate([sinT, -sinT], 0).astype(np.float32)

    ii, jj = np.meshgrid(np.arange(128), np.arange(128), indexing="ij")
    tri = (ii <= jj).astype(np.float32)                 # [k, q] allow k<=q

    tri2 = np.concatenate([np.zeros((128, 128), np.float32), tri], 1)
    at, pj, wd = _npdt("attne"), _npdt("proj"), _npdt("wo")
    common = {
        "c2": c2, "g": gtab, "tri": tri.astype(at), "tri2": tri2.astype(at),
        "ones128": np.ones((128, 1), at),
        "onesrow": np.ones((1, 128), np.float32),
        "one1": np.ones((1, 1), at),
        "onerow512": np.ones((1, 512), at),
        "idn": np.eye(128, dtype=at),
    }
    import ml_dtypes
    common.update({
        "tri_b": tri.astype(ml_dtypes.bfloat16),
        "tri2_b": tri2.astype(ml_dtypes.bfloat16),
        "ones128_b": np.ones((128, 1), ml_dtypes.bfloat16),
        "one1_b": np.ones((1, 1), ml_dtypes.bfloat16),
        "onerow512_b": np.ones((1, 512), ml_dtypes.bfloat16),
    })
    in_maps = []
    for core in range(8):
        b, t = divmod(core, TP)
        in_maps.append({
            "xT": np.ascontiguousarray(x[b].T).astype(pj),
            "wq": np.ascontiguousarray(wq_p[:, t * QF:(t + 1) * QF]).astype(pj),
            "wk": np.ascontiguousarray(wk_p[:, t * KF:(t + 1) * KF]).astype(pj),
            "wv": np.ascontiguousarray(wv[:, t * KF:(t + 1) * KF]).astype(pj),
            "wo": np.ascontiguousarray(wo[t * QF:(t + 1) * QF, :]).astype(wd),
            **common,
        })
    return in_maps


def kernel(x, freqs_cos, freqs_sin, wq, wk, wv, wo, _trace=False):
    in_maps = _host_prep(np.asarray(x, np.float32),
                         np.asarray(freqs_cos, np.float32),
                         np.asarray(freqs_sin, np.float32),
                         np.asarray(wq, np.float32), np.asarray(wk, np.float32),
                         np.asarray(wv, np.float32), np.asarray(wo, np.float32))
    if "nc" not in _CACHE:
        _CACHE["nc"] = _build()
    res = bass_utils.run_bass_kernel_spmd(_CACHE["nc"], in_maps, list(range(8)),
                                          trace=_trace)
    _CACHE["last_result"] = res
    out = np.zeros((B, S, D), np.float32)
    for core in range(8):
        b = core // TP
        out[b] += res.results[core]["out"]
    return out

